# revision 3
# baseline (speedup 1.0000x reference)
"""Trainium2 Bass kernel for nn_Classifier (GNN edge-MLP link predictor).

Computes, for E candidate edges:
    out[e] = W2 . relu( x_nc[i0[e]] @ W1[:H] + x_pr[i1[e]] @ W1[H:] + b1 ) + b2

Strategy (8 NeuronCores, data-parallel over edges):
  Phase 1 (per core): precompute per-node partial activations
      a[n] = x_nc[n] @ W1[:H] + b1/2,   b[n] = x_pr[n] @ W1[H:] + b1/2
  as fp16 [N, 128] row-major tables in DRAM scratch (tensor engine,
  stationary = transposed x chunks, moving = W1 half).

  Phase 2: edges sharded 125k/core. Per tile of T edges, two
  NON-transpose dma_gathers (edge-major layout [128 edges, 128 feat])
  pull a[i0] and b[i1] rows. Descriptor generation is the bottleneck
  engine (GpSimd/Pool SWDGE), so gathers are spread round-robin over 4
  SWDGE queues, which the Q7 cluster executes on distinct CPU pairs
  concurrently (~4x). Transpose-mode gathers cannot do this (shared
  xbar state corrupts concurrent streams); edge-major avoids the xbar,
  and the MLP becomes elementwise + a free-axis reduction:
      h = relu(gA + gB); out = reduce_f(h * w2) + b2
  on DVE (3 passes/tile) + scalar engine (bias), no per-edge matmul.

Output lands as [128, G] per tile (edge g*128+p at partition p, col g);
the host transposes back. fp16 tables + fp16 elementwise, fp32 reduce:
rel err ~1e-3 vs fp32 reference.
"""

import numpy as np
import ml_dtypes

import concourse.bass as bass
import concourse.tile as tile
from concourse import bacc, mybir
from concourse import bass_utils

F32 = mybir.dt.float32
FP16 = mybir.dt.float16
BF16 = mybir.dt.bfloat16
I16 = mybir.dt.int16

N_CORES = 8
H = 128

# Full-problem geometry (hardcoded per the task contract).
E_TOTAL = 1_000_000
N_NODES = 20_000
NP = 20_480  # node tables padded to 40 chunks of 512

T_MAIN = 8192


def _tiles_for(e_core: int, t_main: int):
    """Tile sizes (multiples of 128) covering e_core with minimal padding."""
    n_full = e_core // t_main
    rem = e_core - n_full * t_main
    tiles = [t_main] * n_full
    if rem:
        tiles.append(((rem + 127) // 128) * 128)
    return tiles


def _build(tiles: list, reps: int = 1):
    e_pad = sum(tiles)
    assert all(t % 128 == 0 and t <= T_MAIN for t in tiles)
    g_tot = e_pad // 128

    nc = bacc.Bacc(
        "TRN2",
        target_bir_lowering=False,
        debug=False,
        num_devices=N_CORES,
        num_swdge_queues=4,
    )

    xt_nc = nc.dram_tensor("xt_nc", [H, NP], BF16, kind="ExternalInput").ap()
    xt_pr = nc.dram_tensor("xt_pr", [H, NP], BF16, kind="ExternalInput").ap()
    idx0 = nc.dram_tensor("idx0", [16, e_pad // 16], I16, kind="ExternalInput").ap()
    idx1 = nc.dram_tensor("idx1", [16, e_pad // 16], I16, kind="ExternalInput").ap()
    w1nc = nc.dram_tensor("w1nc", [H, H], BF16, kind="ExternalInput").ap()
    w1pr = nc.dram_tensor("w1pr", [H, H], BF16, kind="ExternalInput").ap()
    hb1r = nc.dram_tensor("hb1r", [1, 512], F32, kind="ExternalInput").ap()
    w2r = nc.dram_tensor("w2r", [1, T_MAIN], FP16, kind="ExternalInput").ap()
    b2 = nc.dram_tensor("b2", [1, 1], F32, kind="ExternalInput").ap()
    out = nc.dram_tensor("out", [128, g_tot], F32, kind="ExternalOutput").ap()

    a_tbl = nc.dram_tensor("a_tbl", [NP, H], FP16, kind="Internal").ap()
    b_tbl = nc.dram_tensor("b_tbl", [NP, H], FP16, kind="Internal").ap()

    relu_max = mybir.AluOpType.max
    add_op = mybir.AluOpType.add
    mult_op = mybir.AluOpType.mult
    bypass = mybir.AluOpType.bypass
    ident = mybir.ActivationFunctionType.Identity

    with tile.TileContext(nc) as tc:
        with (
            tc.tile_pool(name="const", bufs=1) as cpool,
            tc.tile_pool(name="idx", bufs=1) as ipool,
            tc.tile_pool(name="x", bufs=3) as xpool,
            tc.tile_pool(name="ao", bufs=3) as apool,
            tc.tile_pool(name="gather", bufs=2) as gpool,
            tc.tile_pool(name="h", bufs=2) as hpool,
            tc.tile_pool(name="stage", bufs=2) as spool,
            tc.tile_pool(name="ps", bufs=4, space="PSUM") as pspool,
        ):
            # ---- constants ----
            w1nc_sb = cpool.tile([H, H], BF16, tag="w1nc")
            nc.sync.dma_start(w1nc_sb[:], w1nc[:])
            w1pr_sb = cpool.tile([H, H], BF16, tag="w1pr")
            nc.sync.dma_start(w1pr_sb[:], w1pr[:])
            hb1_row = cpool.tile([1, 512], F32, tag="hb1row")
            nc.sync.dma_start(hb1_row[:], hb1r[:])
            w2_row = cpool.tile([1, T_MAIN], FP16, tag="w2row")
            nc.sync.dma_start(w2_row[:], w2r[:])
            b2_sb = cpool.tile([1, 1], F32, tag="b2")
            nc.sync.dma_start(b2_sb[:], b2[:])

            hb1_rep = cpool.tile([128, 512], F32, tag="hb1rep")
            nc.gpsimd.partition_broadcast(hb1_rep[:], hb1_row[:])
            w2_rep = cpool.tile([128, T_MAIN], FP16, tag="w2rep")
            nc.gpsimd.partition_broadcast(w2_rep[:], w2_row[:])
            b2_rep = cpool.tile([128, 1], F32, tag="b2rep")
            nc.gpsimd.partition_broadcast(b2_rep[:], b2_sb[:])

            # ---- indices: [16, N] wrapped, replicated across partition groups
            idx0_sb = ipool.tile([128, e_pad // 16], I16, tag="idx0")
            idx1_sb = ipool.tile([128, e_pad // 16], I16, tag="idx1")
            for k in range(8):
                nc.sync.dma_start(idx0_sb[16 * k : 16 * (k + 1), :], idx0[:])
                nc.sync.dma_start(idx1_sb[16 * k : 16 * (k + 1), :], idx1[:])

            # ---- phase 1: a/b node tables ----
            for tbl, w1_sb, xt in (
                (a_tbl, w1nc_sb, xt_nc),
                (b_tbl, w1pr_sb, xt_pr),
            ):
                tbl_v = tbl.rearrange("(b p) f -> p b f", p=128)
                for c in range(NP // 512):
                    sl = slice(c * 512, (c + 1) * 512)
                    xc = xpool.tile([H, 512], BF16, tag="xc")
                    nc.sync.dma_start(xc[:], xt[:, sl])
                    ps = pspool.tile([128, 512], F32, tag="ps")
                    for k in range(4):
                        ks = slice(k * 128, (k + 1) * 128)
                        nc.tensor.matmul(
                            ps[:, ks], xc[:, ks], w1_sb[:], start=True, stop=True
                        )
                    ao = apool.tile([128, 512], FP16, tag="ao")
                    nc.vector.scalar_tensor_tensor(
                        ao[:], ps[:], 0.0, hb1_rep[:], bypass, add_op
                    )
                    nc.sync.dma_start(
                        tbl_v[:, 4 * c : 4 * c + 4, :],
                        ao[:].rearrange("p (k f) -> p k f", k=4),
                    )

            # ---- phase 2: edge loop ----
            seq = [t for _ in range(reps) for t in enumerate(tiles)]
            for i, (ti, t) in enumerate(seq):
                g = t // 128
                g0 = sum(tiles[:ti]) // 128
                c0 = sum(tiles[:ti]) // 16
                ic = t // 16

                gA = gpool.tile([128, T_MAIN], FP16, tag="gA")
                nc.gpsimd.dma_gather(
                    gA[:, :t].rearrange("p (g f) -> p g f", f=H),
                    a_tbl,
                    idx0_sb[:, c0 : c0 + ic],
                    t,
                    t,
                    H,
                    transpose=False,
                    single_packet=False,
                    queue_num=(2 * i) % 4,
                )
                gB = gpool.tile([128, T_MAIN], FP16, tag="gB")
                nc.gpsimd.dma_gather(
                    gB[:, :t].rearrange("p (g f) -> p g f", f=H),
                    b_tbl,
                    idx1_sb[:, c0 : c0 + ic],
                    t,
                    t,
                    H,
                    transpose=False,
                    single_packet=False,
                    queue_num=(2 * i + 1) % 4,
                )

                h = hpool.tile([128, T_MAIN], FP16, tag="h")
                nc.vector.tensor_tensor(h[:, :t], gA[:, :t], gB[:, :t], add_op)
                m = hpool.tile([128, T_MAIN], FP16, tag="m")
                nc.vector.scalar_tensor_tensor(
                    m[:, :t], h[:, :t], 0.0, w2_rep[:, :t], relu_max, mult_op
                )
                red = spool.tile([128, T_MAIN // 128], F32, tag="red")
                nc.vector.tensor_reduce(
                    red[:, :g],
                    m[:, :t].rearrange("p (g f) -> p g f", f=H),
                    mybir.AxisListType.X,
                    add_op,
                )
                stage = spool.tile([128, T_MAIN // 128], F32, tag="stage")
                nc.scalar.activation(stage[:, :g], red[:, :g], ident, bias=b2_rep[:])
                nc.sync.dma_start(out[:, g0 : g0 + g], stage[:, :g])

    nc.compile()
    return nc


# ---------------------------------------------------------------------------
# Host-side wrapper
# ---------------------------------------------------------------------------

_CACHE: dict = {}


def _wrap_idx(idx: np.ndarray, e_pad: int) -> np.ndarray:
    """int16 [16, e_pad//16] with index i at [i % 16, i // 16]."""
    pad = np.zeros(e_pad, np.int16)
    pad[: idx.shape[0]] = idx.astype(np.int16)
    return np.ascontiguousarray(pad.reshape(e_pad // 16, 16).T)


def _get_program(tiles):
    key = tuple(tiles)
    if key not in _CACHE:
        _CACHE[key] = _build(list(tiles))
    return _CACHE[key]


def kernel(
    x_ncRNA: np.ndarray,
    x_Protein: np.ndarray,
    edge_label_index: np.ndarray,
    W1: np.ndarray,
    b1: np.ndarray,
    W2: np.ndarray,
    b2: np.ndarray,
    _trace: bool = False,
) -> np.ndarray:
    E = edge_label_index.shape[1]
    n_nodes = x_ncRNA.shape[0]
    assert E % N_CORES == 0 and n_nodes <= NP
    e_core = E // N_CORES
    tiles = _tiles_for(e_core, T_MAIN)
    e_pad = sum(tiles)
    g_tot = e_pad // 128

    nc = _get_program(tiles)

    def prep_xt(x):
        xt = np.zeros((H, NP), ml_dtypes.bfloat16)
        xt[:, :n_nodes] = x.T.astype(ml_dtypes.bfloat16)
        return np.ascontiguousarray(xt)

    xt_nc = prep_xt(x_ncRNA)
    xt_pr = prep_xt(x_Protein)
    w1nc = np.ascontiguousarray(W1[:H].astype(ml_dtypes.bfloat16))
    w1pr = np.ascontiguousarray(W1[H:].astype(ml_dtypes.bfloat16))
    hb1r = np.ascontiguousarray(
        np.tile(0.5 * b1.astype(np.float32), 4).reshape(1, 512)
    )
    w2r = np.ascontiguousarray(
        np.tile(W2[:, 0].astype(np.float16), T_MAIN // H).reshape(1, T_MAIN)
    )
    b2_ = np.ascontiguousarray(b2.reshape(1, 1).astype(np.float32))

    in_maps = []
    for c in range(N_CORES):
        sl = slice(c * e_core, (c + 1) * e_core)
        in_maps.append(
            {
                "xt_nc": xt_nc,
                "xt_pr": xt_pr,
                "idx0": _wrap_idx(np.asarray(edge_label_index[0, sl]), e_pad),
                "idx1": _wrap_idx(np.asarray(edge_label_index[1, sl]), e_pad),
                "w1nc": w1nc,
                "w1pr": w1pr,
                "hb1r": hb1r,
                "w2r": w2r,
                "b2": b2_,
            }
        )

    res = bass_utils.run_bass_kernel_spmd(
        nc, in_maps, core_ids=list(range(N_CORES)), trace=_trace
    )
    out = np.empty(E, np.float32)
    for c in range(N_CORES):
        # out[p, g] = edge g*128 + p of this core
        flat = res.results[c]["out"].T.reshape(-1)
        out[c * e_core : (c + 1) * e_core] = flat[:e_core]
    kernel._last_results = res
    return out
